# revision 16
# baseline (speedup 1.0000x reference)
"""Trainium2 Bass kernel for nn_EnhancedAdaptiveRecursiveCell.

Strategy (8 NeuronCores, data-parallel over batch):
  * seq_len==1 => softmax(scores)==1 exactly => attention output == v_proj;
    the q/k projections are dead code and skipped.
  * ComplexLinear via Karatsuba (3 real matmuls):
      W1 = Wr, W2 = Wi - Wr, W3 = Wr + Wi   (fp16)
      k1 = (xr+xi)@W1, k2 = xr@W2, k3 = xi@W3
      out_r = k1 - k3, out_i = k1 + k2
  * fp16 matmuls, near-fp32 accuracy:
      - activations split exactly x = xh + xl (fp16 pair), K-concat
      - weight residual Wl = W - fp16(W) kept in fp16 (denormal range,
        honored by the PE); correction term xh@Wl accumulated too.
  * Activations transposed [d, rows]; weights stream as the stationary
    operand from a tiled DRAM layout [m][p][k][c] (4KB-contiguous loads).
  * z input is split in natural layout then loaded transposed via the
    XBAR dma transpose; zr/zi outputs are PE-transposed before store so
    every DMA runs with large contiguous elements.
"""

import sys

sys.path.insert(0, "/opt/trn_rl_repo")

import numpy as np

N_CORES = 8
B = 16384
D = 2048
BL = B // N_CORES
P = 128
KT = D // P
MT = D // P
NB = 512
NPASS = BL // NB
EPS = 1e-6

_CACHE = {}


def _build():
    import concourse.bass as bass
    import concourse.tile as tile
    import concourse.masks as masks
    from concourse import bacc, mybir
    from contextlib import ExitStack

    F32 = mybir.dt.float32
    F16 = mybir.dt.float16
    OP = mybir.AluOpType
    AF = mybir.ActivationFunctionType

    nc = bacc.Bacc("TRN2", target_bir_lowering=False, debug=False,
                   num_devices=N_CORES)

    z_real = nc.dram_tensor("z_real", [BL, D], F32, kind="ExternalInput")
    z_imag = nc.dram_tensor("z_imag", [BL, D], F32, kind="ExternalInput")
    Wn = {}
    for nm in ["Wlr", "Wli", "Wvr", "Wvi", "Wor", "Woi"]:
        Wn[nm] = nc.dram_tensor(nm, [D, D], F32, kind="ExternalInput")
    n_scale = nc.dram_tensor("n_scale", [D], F32, kind="ExternalInput")
    n_shift = nc.dram_tensor("n_shift", [D], F32, kind="ExternalInput")
    mr_bias = nc.dram_tensor("mr_bias", [D], F32, kind="ExternalInput")
    Wh = nc.dram_tensor("Wh", [2 * D, 1], F32, kind="ExternalInput")
    bh = nc.dram_tensor("bh", [1], F32, kind="ExternalInput")
    Ws = nc.dram_tensor("Ws", [2 * D, 3], F32, kind="ExternalInput")
    bs = nc.dram_tensor("bs", [3], F32, kind="ExternalInput")

    zr_out = nc.dram_tensor("zr_out", [BL, D], F32, kind="ExternalOutput")
    zi_out = nc.dram_tensor("zi_out", [BL, D], F32, kind="ExternalOutput")
    halt_out = nc.dram_tensor("halt_out", [BL, 1], F32, kind="ExternalOutput")
    stack_out = nc.dram_tensor("stack_out", [BL, 3], F32, kind="ExternalOutput")

    with tile.TileContext(nc) as tc:
        ctx = ExitStack()
        with ctx:
            dram = ctx.enter_context(tc.tile_pool(name="dram", bufs=1, space="DRAM"))

            # z pairs in NATURAL layout (contiguous writes, xbar-transpose reads)
            zN = {}
            for k in ("r", "i", "s"):
                zN[k] = (dram.tile([BL, D], F16, name=f"zN_{k}_h"),
                         dram.tile([BL, D], F16, name=f"zN_{k}_l"))
            # aT/vT pairs in transposed layout [d, rows]
            aT = {k: (dram.tile([D, BL], F16, name=f"aT_{k}_h"),
                      dram.tile([D, BL], F16, name=f"aT_{k}_l"))
                  for k in ("r", "i", "s")}
            vT = {k: (dram.tile([D, BL], F16, name=f"vT_{k}_h"),
                      dram.tile([D, BL], F16, name=f"vT_{k}_l"))
                  for k in ("r", "i", "s")}
            # tiled weight combos [m][p][k][c] and their fp16 residuals
            combos, wlcombos = {}, {}
            for cl in ("l", "v", "o"):
                for j in (1, 2, 3):
                    combos[(cl, j)] = dram.tile([MT, P, KT, P], F16,
                                                name=f"W{cl}{j}")
                    wlcombos[(cl, j)] = dram.tile([MT, P, KT, P], F16,
                                                  name=f"Wl{cl}{j}")

            small = ctx.enter_context(tc.tile_pool(name="small", bufs=1))
            rhsp = ctx.enter_context(tc.tile_pool(name="rhs", bufs=1))
            wmp = ctx.enter_context(tc.tile_pool(name="wm", bufs=2))
            wlp = ctx.enter_context(tc.tile_pool(name="wl", bufs=1))
            pers = ctx.enter_context(tc.tile_pool(name="pers", bufs=1))
            work = ctx.enter_context(tc.tile_pool(name="work", bufs=9))
            outp = ctx.enter_context(tc.tile_pool(name="outp", bufs=1))
            psum = ctx.enter_context(tc.tile_pool(name="psum", bufs=2, space="PSUM"))

            def rtile(tag, shape, dtype, name):
                return rhsp.tile(shape, dtype, name=name, tag=tag)

            def ptile(tag, shape, dtype, name):
                return pers.tile(shape, dtype, name=name, tag=tag)

            def tmp(shape, dtype, name):
                return work.tile(shape, dtype, name=name, tag="tmp")

            # ---------- small resident constants ----------
            ones16 = small.tile([1, P], F16, name="ones16")
            nc.vector.memset(ones16[:], 1.0)
            onesc = small.tile([P, 1], F16, name="onesc")
            nc.vector.memset(onesc[:], 1.0)
            ident = small.tile([P, P], F32, name="ident")
            masks.make_identity(nc, ident[:])
            scale_t = small.tile([P, MT], F32, name="scale_t")
            nc.sync.dma_start(scale_t[:], n_scale.ap().rearrange("(m p) -> p m", p=P))
            shift_t = small.tile([P, MT], F32, name="shift_t")
            nc.sync.dma_start(shift_t[:], n_shift.ap().rearrange("(m p) -> p m", p=P))
            mrb_t = small.tile([P, MT], F32, name="mrb_t")
            nc.sync.dma_start(mrb_t[:], mr_bias.ap().rearrange("(m p) -> p m", p=P))
            biaseps_t = small.tile([P, MT], F32, name="biaseps_t")
            nc.vector.tensor_scalar_add(biaseps_t[:], mrb_t[:], float(EPS))
            wh16 = small.tile([P, 2 * KT], F16, name="wh16")
            ws16 = small.tile([P, 2 * KT, 3], F16, name="ws16")
            bh_t = small.tile([1, 1], F32, name="bh_t")
            nc.sync.dma_start(bh_t[:], bh.ap().rearrange("a -> () a"))
            bs_t = small.tile([1, 3], F32, name="bs_t")
            nc.sync.dma_start(bs_t[:], bs.ap().rearrange("a -> () a"))
            wh32 = tmp([P, 2 * KT], F32, "wh32")
            nc.sync.dma_start(wh32[:], Wh.ap().rearrange("(j p) c -> p (j c)", p=P))
            nc.scalar.copy(wh16[:], wh32[:])
            ws32 = tmp([P, 2 * KT, 3], F32, "ws32")
            nc.sync.dma_start(ws32[:], Ws.ap().rearrange("(j p) c -> p j c", p=P))
            nc.scalar.copy(ws16[:], ws32[:])

            # ---------- P0: split z into fp16 pairs (natural layout) ----------
            for rt in range(KT):
                for q in range(4):
                    qsl = slice(q * NB, (q + 1) * NB)
                    psl = slice(rt * P, (rt + 1) * P)
                    zr32 = tmp([P, NB], F32, "zr32")
                    nc.sync.dma_start(zr32[:], z_real.ap()[psl, qsl])
                    zi32 = tmp([P, NB], F32, "zi32")
                    nc.sync.dma_start(zi32[:], z_imag.ap()[psl, qsl])
                    zs32 = tmp([P, NB], F32, "zs32")
                    nc.vector.tensor_tensor(zs32[:], zr32[:], zi32[:], op=OP.add)
                    for key, src in (("r", zr32), ("i", zi32), ("s", zs32)):
                        h16 = outp.tile([P, NB], F16, name=f"zh_{key}",
                                        tag=f"h_{key}")
                        nc.scalar.copy(h16[:], src[:])
                        l16 = outp.tile([P, NB], F16, name=f"zl_{key}",
                                        tag=f"l_{key}")
                        nc.vector.scalar_tensor_tensor(
                            l16[:], src[:], 1.0, h16[:],
                            op0=OP.mult, op1=OP.subtract)
                        hD, lD = zN[key]
                        nc.sync.dma_start(hD[:][psl, qsl], h16[:])
                        nc.sync.dma_start(lD[:][psl, qsl], l16[:])

            # ---------- P0b: fp16 Karatsuba combos + residuals, tiled ----------
            def combo_store(dst, t16, j, c):
                # sbuf [128, NB] (= [128, 4 m-tiles, 128]) -> [m][p][k][c] tiled
                nc.sync.dma_start(
                    dst[:][4 * c:4 * (c + 1), :, j, :].rearrange("m p c -> p m c"),
                    t16[:].rearrange("p (m c) -> p m c", m=4))

            for cl, wr_n, wi_n in (("l", "Wlr", "Wli"), ("v", "Wvr", "Wvi"),
                                   ("o", "Wor", "Woi")):
                for j in range(KT):
                    for c in range(4):
                        csl = slice(c * NB, (c + 1) * NB)
                        psl = slice(j * P, (j + 1) * P)
                        wr32 = tmp([P, NB], F32, "wr32")
                        nc.sync.dma_start(wr32[:], Wn[wr_n].ap()[psl, csl])
                        wi32 = tmp([P, NB], F32, "wi32")
                        nc.sync.dma_start(wi32[:], Wn[wi_n].ap()[psl, csl])
                        v2 = tmp([P, NB], F32, "v2")
                        nc.vector.tensor_tensor(v2[:], wi32[:], wr32[:],
                                                op=OP.subtract)
                        v3 = tmp([P, NB], F32, "v3")
                        nc.vector.tensor_tensor(v3[:], wi32[:], wr32[:], op=OP.add)
                        for ji, v32 in ((1, wr32), (2, v2), (3, v3)):
                            w16 = outp.tile([P, NB], F16, name=f"w{ji}",
                                            tag=f"h_{'ris'[ji-1]}")
                            nc.scalar.copy(w16[:], v32[:])
                            wl16 = outp.tile([P, NB], F16, name=f"wl{ji}",
                                             tag=f"l_{'ris'[ji-1]}")
                            nc.vector.scalar_tensor_tensor(
                                wl16[:], v32[:], 1.0, w16[:],
                                op0=OP.mult, op1=OP.subtract)
                            combo_store(combos[(cl, ji)], w16, j, c)
                            combo_store(wlcombos[(cl, ji)], wl16, j, c)

            # ---------- helpers ----------
            def load_rhs_z(key, rb):
                hD, lD = zN[key]
                th = rtile(f"rh_{key}", [P, KT, NB], F16, f"rh{key}")
                tl = rtile(f"rl_{key}", [P, KT, NB], F16, f"rl{key}")
                for k in range(KT):
                    nc.sync.dma_start_transpose(
                        th[:, k, :],
                        hD[:][rb * NB:(rb + 1) * NB, k * P:(k + 1) * P])
                    nc.sync.dma_start_transpose(
                        tl[:, k, :],
                        lD[:][rb * NB:(rb + 1) * NB, k * P:(k + 1) * P])
                return th, tl

            def load_rhs_t(srcpair, key, rb):
                hD, lD = srcpair
                th = rtile(f"rh_{key}", [P, KT, NB], F16, f"rh{key}")
                nc.sync.dma_start(
                    th[:], hD[:].rearrange("(j p) r -> p j r", p=P)
                    [:, :, rb * NB:(rb + 1) * NB])
                tl = rtile(f"rl_{key}", [P, KT, NB], F16, f"rl{key}")
                nc.sync.dma_start(
                    tl[:], lD[:].rearrange("(j p) r -> p j r", p=P)
                    [:, :, rb * NB:(rb + 1) * NB])
                return th, tl

            def gemm3(cl, m, rhs):
                ps = {}
                for ji, rkey in ((1, "s"), (2, "r"), (3, "i")):
                    wm = wmp.tile([P, KT, P], F16, name=f"wm{ji}_{m}", tag=f"wm{ji}")
                    nc.sync.dma_start(wm[:], combos[(cl, ji)][:][m])
                    wlm = wlp.tile([P, KT, P], F16, name=f"wl{ji}_{m}", tag=f"wl{ji}")
                    nc.sync.dma_start(wlm[:], wlcombos[(cl, ji)][:][m])
                    pt = psum.tile([P, NB], F32, name=f"k{ji}_{m}", tag=f"k{ji}")
                    th, tl = rhs[rkey]
                    n = 0
                    ntot = 3 * KT
                    for half in (th, tl):
                        for j in range(KT):
                            nc.tensor.matmul(pt[:], wm[:, j, :], half[:, j, :],
                                             start=(n == 0), stop=(n == ntot - 1),
                                             skip_group_check=True)
                            n += 1
                    for j in range(KT):
                        nc.tensor.matmul(pt[:], wlm[:, j, :], th[:, j, :],
                                         start=False, stop=(n == ntot - 1),
                                         skip_group_check=True)
                        n += 1
                    ps[ji] = pt
                return ps

            # ---------- P1: l-proj + ComplexLayerNorm + ModReLU ----------
            for rb in range(NPASS):
                rhs = {k: load_rhs_z(k, rb) for k in ("r", "i", "s")}
                lr16 = ptile("lr16", [P, MT, NB], F16, "lr16")
                li16 = ptile("li16", [P, MT, NB], F16, "li16")
                st_m = psum.tile([1, NB], F32, name="st_m", tag="aux")
                st_s = psum.tile([1, NB], F32, name="st_s", tag="aux")
                s16s = []
                m16s = []
                for m in range(MT):
                    ps = gemm3("l", m, rhs)
                    if m > 0:
                        nc.tensor.matmul(st_m[:], onesc[:], m16s[m - 1][:],
                                         start=(m - 1 == 0), stop=False,
                                         skip_group_check=True)
                        nc.tensor.matmul(st_s[:], onesc[:], s16s[m - 1][:],
                                         start=(m - 1 == 0), stop=False,
                                         skip_group_check=True)
                    k1sb = tmp([P, NB], F32, "k1sb")
                    nc.vector.tensor_copy(k1sb[:], ps[1][:])
                    nc.vector.tensor_tensor(lr16[:, m, :], k1sb[:], ps[3][:],
                                            op=OP.subtract)
                    nc.vector.tensor_tensor(li16[:, m, :], k1sb[:], ps[2][:],
                                            op=OP.add)
                    s32 = tmp([P, NB], F32, "s32")
                    nc.vector.tensor_tensor(s32[:], lr16[:, m, :], lr16[:, m, :],
                                            op=OP.mult)
                    t2 = tmp([P, NB], F32, "t2")
                    nc.vector.tensor_tensor(t2[:], li16[:, m, :], li16[:, m, :],
                                            op=OP.mult)
                    nc.vector.tensor_tensor(s32[:], s32[:], t2[:], op=OP.add)
                    r0 = tmp([P, NB], F32, "r0")
                    nc.scalar.activation(r0[:], s32[:], AF.Sqrt)
                    m16 = work.tile([P, NB], F16, name="m16", tag="st16", bufs=4)
                    nc.vector.tensor_scalar_add(m16[:], r0[:], float(EPS))
                    s16 = work.tile([P, NB], F16, name="s16", tag="st16", bufs=4)
                    nc.vector.tensor_copy(s16[:], s32[:])
                    s16s.append(s16)
                    m16s.append(m16)
                nc.tensor.matmul(st_m[:], onesc[:], m16s[MT - 1][:],
                                 start=False, stop=True, skip_group_check=True)
                nc.tensor.matmul(st_s[:], onesc[:], s16s[MT - 1][:],
                                 start=False, stop=True, skip_group_check=True)
                smsb = tmp([1, NB], F32, "smsb")
                nc.vector.tensor_copy(smsb[:], st_m[:])
                sssb = tmp([1, NB], F32, "sssb")
                nc.vector.tensor_copy(sssb[:], st_s[:])
                mean = tmp([1, NB], F32, "mean")
                nc.vector.tensor_scalar_mul(mean[:], smsb[:], 1.0 / D)
                tst = tmp([1, NB], F32, "tst")
                nc.vector.tensor_tensor(tst[:], mean[:], smsb[:], op=OP.mult)
                var = tmp([1, NB], F32, "var")
                nc.vector.tensor_tensor(var[:], sssb[:], tst[:], op=OP.subtract)
                nc.vector.tensor_scalar(var[:], var[:], 1.0 / (D - 1), float(EPS),
                                        op0=OP.mult, op1=OP.add)
                sq = tmp([1, NB], F32, "sq")
                nc.scalar.activation(sq[:], var[:], AF.Sqrt)
                rstd = tmp([1, NB], F32, "rstd")
                nc.vector.reciprocal(rstd[:], sq[:])
                brow_m = tmp([1, NB], F16, "brow_m")
                nc.vector.tensor_copy(brow_m[:], mean[:])
                brow_s = tmp([1, NB], F16, "brow_s")
                nc.vector.tensor_copy(brow_s[:], rstd[:])
                mean_b = psum.tile([P, NB], F32, name="mean_b", tag="aux")
                nc.tensor.matmul(mean_b[:], ones16[:], brow_m[:],
                                 start=True, stop=True, skip_group_check=True)
                rstd_b = psum.tile([P, NB], F32, name="rstd_b", tag="aux")
                nc.tensor.matmul(rstd_b[:], ones16[:], brow_s[:],
                                 start=True, stop=True, skip_group_check=True)
                for m in range(MT):
                    s32 = tmp([P, NB], F32, "s32a")
                    nc.vector.tensor_tensor(s32[:], lr16[:, m, :], lr16[:, m, :],
                                            op=OP.mult)
                    t2 = tmp([P, NB], F32, "t2a")
                    nc.vector.tensor_tensor(t2[:], li16[:, m, :], li16[:, m, :],
                                            op=OP.mult)
                    nc.vector.tensor_tensor(s32[:], s32[:], t2[:], op=OP.add)
                    r0 = tmp([P, NB], F32, "r0a")
                    nc.scalar.activation(r0[:], s32[:], AF.Sqrt)
                    mg = tmp([P, NB], F32, "mga")
                    nc.vector.tensor_scalar_add(mg[:], r0[:], float(EPS))
                    rmag = tmp([P, NB], F32, "rmag")
                    nc.vector.reciprocal(rmag[:], mg[:])
                    nmt = tmp([P, NB], F32, "nmt")
                    nc.vector.tensor_tensor(nmt[:], mg[:], mean_b[:], op=OP.subtract)
                    nc.vector.tensor_tensor(nmt[:], nmt[:], rstd_b[:], op=OP.mult)
                    nc.vector.tensor_scalar(nmt[:], nmt[:], scale_t[:, m:m + 1],
                                            shift_t[:, m:m + 1],
                                            op0=OP.mult, op1=OP.add)
                    Ft = tmp([P, NB], F32, "Ft")
                    nc.vector.tensor_tensor(Ft[:], nmt[:], rmag[:], op=OP.mult)
                    q = tmp([P, NB], F32, "q")
                    nc.vector.tensor_tensor(q[:], r0[:], rmag[:], op=OP.mult)
                    absn = tmp([P, NB], F32, "absn")
                    nc.scalar.activation(absn[:], nmt[:], AF.Abs)
                    norm0 = tmp([P, NB], F32, "norm0")
                    nc.vector.tensor_tensor(norm0[:], absn[:], q[:], op=OP.mult)
                    relu = tmp([P, NB], F32, "relu")
                    nc.scalar.activation(relu[:], norm0[:], AF.Relu,
                                         bias=biaseps_t[:, m:m + 1])
                    normp = tmp([P, NB], F32, "normp")
                    nc.vector.tensor_scalar_add(normp[:], norm0[:], float(EPS))
                    rn = tmp([P, NB], F32, "rn")
                    nc.vector.reciprocal(rn[:], normp[:])
                    nc.vector.tensor_tensor(Ft[:], Ft[:], relu[:], op=OP.mult)
                    nc.vector.tensor_tensor(Ft[:], Ft[:], rn[:], op=OP.mult)
                    tr = tmp([P, NB], F32, "tr")
                    nc.vector.tensor_tensor(tr[:], Ft[:], lr16[:, m, :], op=OP.mult)
                    ti = tmp([P, NB], F32, "ti")
                    nc.vector.tensor_tensor(ti[:], Ft[:], li16[:, m, :], op=OP.mult)
                    ts_ = tmp([P, NB], F32, "ts_")
                    nc.vector.tensor_tensor(ts_[:], tr[:], ti[:], op=OP.add)
                    for key, src in (("r", tr), ("i", ti), ("s", ts_)):
                        h16 = outp.tile([P, NB], F16, name=f"ah_{key}",
                                        tag=f"h_{key}")
                        nc.scalar.copy(h16[:], src[:])
                        l16 = outp.tile([P, NB], F16, name=f"al_{key}",
                                        tag=f"l_{key}")
                        nc.vector.scalar_tensor_tensor(
                            l16[:], src[:], 1.0, h16[:], op0=OP.mult,
                            op1=OP.subtract)
                        hD, lD = aT[key]
                        nc.sync.dma_start(
                            hD[:][m * P:(m + 1) * P, rb * NB:(rb + 1) * NB], h16[:])
                        nc.sync.dma_start(
                            lD[:][m * P:(m + 1) * P, rb * NB:(rb + 1) * NB], l16[:])

            # ---------- P2: v-proj ----------
            for rb in range(NPASS):
                rhs = {k: load_rhs_t(aT[k], k, rb) for k in ("r", "i", "s")}
                for m in range(MT):
                    ps = gemm3("v", m, rhs)
                    k1sb = tmp([P, NB], F32, "k1sbv")
                    nc.vector.tensor_copy(k1sb[:], ps[1][:])
                    vr32 = tmp([P, NB], F32, "vr32")
                    nc.vector.tensor_tensor(vr32[:], k1sb[:], ps[3][:], op=OP.subtract)
                    vi32 = tmp([P, NB], F32, "vi32")
                    nc.vector.tensor_tensor(vi32[:], k1sb[:], ps[2][:], op=OP.add)
                    vs32 = tmp([P, NB], F32, "vs32")
                    nc.vector.tensor_tensor(vs32[:], vr32[:], vi32[:], op=OP.add)
                    for key, src in (("r", vr32), ("i", vi32), ("s", vs32)):
                        h16 = outp.tile([P, NB], F16, name=f"vh_{key}",
                                        tag=f"h_{key}")
                        nc.scalar.copy(h16[:], src[:])
                        l16 = outp.tile([P, NB], F16, name=f"vl_{key}",
                                        tag=f"l_{key}")
                        nc.vector.scalar_tensor_tensor(
                            l16[:], src[:], 1.0, h16[:], op0=OP.mult,
                            op1=OP.subtract)
                        hD, lD = vT[key]
                        nc.sync.dma_start(
                            hD[:][m * P:(m + 1) * P, rb * NB:(rb + 1) * NB], h16[:])
                        nc.sync.dma_start(
                            lD[:][m * P:(m + 1) * P, rb * NB:(rb + 1) * NB], l16[:])

            # ---------- P3: o-proj + transposed outputs + heads ----------
            for rb in range(NPASS):
                rhs = {k: load_rhs_t(vT[k], k, rb) for k in ("r", "i", "s")}
                oh_r = ptile("lr16", [P, MT, NB], F16, "oh_r")
                oh_i = ptile("li16", [P, MT, NB], F16, "oh_i")
                hp = psum.tile([1, NB], F32, name="hp", tag="aux")
                sp = psum.tile([3, NB], F32, name="sp", tag="aux")
                for m in range(MT):
                    ps = gemm3("o", m, rhs)
                    if m > 0:
                        jj = m - 1
                        nc.tensor.matmul(hp[:], wh16[:, jj:jj + 1], oh_r[:, jj, :],
                                         start=(jj == 0), stop=False,
                                         skip_group_check=True)
                        nc.tensor.matmul(hp[:], wh16[:, KT + jj:KT + jj + 1],
                                         oh_i[:, jj, :], start=False, stop=False,
                                         skip_group_check=True)
                        nc.tensor.matmul(sp[:], ws16[:, jj, :], oh_r[:, jj, :],
                                         start=(jj == 0), stop=False,
                                         skip_group_check=True)
                        nc.tensor.matmul(sp[:], ws16[:, KT + jj, :], oh_i[:, jj, :],
                                         start=False, stop=False,
                                         skip_group_check=True)
                    k1sb = tmp([P, NB], F32, "k1sbo")
                    nc.vector.tensor_copy(k1sb[:], ps[1][:])
                    or32 = tmp([P, NB], F32, "or32")
                    nc.vector.tensor_tensor(or32[:], k1sb[:], ps[3][:], op=OP.subtract)
                    oi32 = tmp([P, NB], F32, "oi32")
                    nc.vector.tensor_tensor(oi32[:], k1sb[:], ps[2][:], op=OP.add)
                    nc.scalar.copy(oh_r[:, m, :], or32[:])
                    nc.scalar.copy(oh_i[:, m, :], oi32[:])
                    # PE-transpose each 128x128 block, store rows-major
                    for dst, src, ptag in ((zr_out, or32, "k2"), (zi_out, oi32, "k3")):
                        for b_ in range(NB // P):
                            tp = psum.tile([P, P], F32, name=f"tp_{ptag}",
                                           tag=ptag)
                            nc.tensor.transpose(tp[:], src[:, b_ * P:(b_ + 1) * P],
                                                ident[:])
                            ot = tmp([P, P], F32, f"ot_{ptag}")
                            nc.vector.tensor_copy(ot[:], tp[:])
                            nc.sync.dma_start(
                                dst.ap()[rb * NB + b_ * P:rb * NB + (b_ + 1) * P,
                                         m * P:(m + 1) * P], ot[:])
                jj = MT - 1
                nc.tensor.matmul(hp[:], wh16[:, jj:jj + 1], oh_r[:, jj, :],
                                 start=False, stop=False, skip_group_check=True)
                nc.tensor.matmul(hp[:], wh16[:, KT + jj:KT + jj + 1], oh_i[:, jj, :],
                                 start=False, stop=True, skip_group_check=True)
                nc.tensor.matmul(sp[:], ws16[:, jj, :], oh_r[:, jj, :],
                                 start=False, stop=False, skip_group_check=True)
                nc.tensor.matmul(sp[:], ws16[:, KT + jj, :], oh_i[:, jj, :],
                                 start=False, stop=True, skip_group_check=True)
                hout = tmp([1, NB], F32, "hout")
                nc.scalar.activation(hout[:], hp[:], AF.Sigmoid, bias=bh_t[:])
                nc.sync.dma_start(
                    halt_out.ap()[rb * NB:(rb + 1) * NB, :].rearrange("r c -> c r"),
                    hout[:])
                stsb = tmp([3, NB], F32, "stsb")
                nc.vector.tensor_copy(stsb[:], sp[:])
                flat = work.tile([1, 3, NB], F32, name="flat", tag="flat3", bufs=2)
                for k in range(3):
                    nc.sync.dma_start(flat[:, k, :], stsb[:][k:k + 1, :])
                for k in range(3):
                    nc.vector.tensor_scalar_add(flat[:, k, :], flat[:, k, :],
                                                bs_t[:, k:k + 1])
                mx = tmp([1, NB], F32, "mx")
                nc.vector.tensor_tensor(mx[:], flat[:, 0, :], flat[:, 1, :], op=OP.max)
                nc.vector.tensor_tensor(mx[:], mx[:], flat[:, 2, :], op=OP.max)
                ex = work.tile([1, 3, NB], F32, name="ex", tag="flat3", bufs=2)
                for k in range(3):
                    dk = tmp([1, NB], F32, f"dk{k}")
                    nc.vector.tensor_tensor(dk[:], flat[:, k, :], mx[:], op=OP.subtract)
                    nc.scalar.activation(ex[:, k, :], dk[:], AF.Exp)
                sm = tmp([1, NB], F32, "sm")
                nc.vector.tensor_tensor(sm[:], ex[:, 0, :], ex[:, 1, :], op=OP.add)
                nc.vector.tensor_tensor(sm[:], sm[:], ex[:, 2, :], op=OP.add)
                rs = tmp([1, NB], F32, "rs")
                nc.vector.reciprocal(rs[:], sm[:])
                for k in range(3):
                    nc.vector.tensor_tensor(ex[:, k, :], ex[:, k, :], rs[:],
                                            op=OP.mult)
                nc.sync.dma_start(
                    stack_out.ap()[rb * NB:(rb + 1) * NB, :]
                    .rearrange("r k -> () k r"), ex[:])

    nc.compile()
    return nc


def _get_nc():
    if "nc" not in _CACHE:
        _CACHE["nc"] = _build()
    return _CACHE["nc"]


def kernel(**inputs):
    from concourse.bass_utils import run_bass_kernel_spmd

    nc = _get_nc()
    inp = {k: np.ascontiguousarray(np.asarray(v, dtype=np.float32))
           for k, v in inputs.items()}
    shared = {k: inp[k] for k in
              ["Wlr", "Wli", "Wvr", "Wvi", "Wor", "Woi",
               "n_scale", "n_shift", "mr_bias", "Wh", "bh", "Ws", "bs"]}
    in_maps = []
    for c in range(N_CORES):
        m = dict(shared)
        m["z_real"] = inp["z_real"][c * BL:(c + 1) * BL]
        m["z_imag"] = inp["z_imag"][c * BL:(c + 1) * BL]
        in_maps.append(m)
    res = run_bass_kernel_spmd(nc, in_maps, core_ids=list(range(N_CORES)),
                               trace=False)
    zr = np.concatenate([res.results[c]["zr_out"] for c in range(N_CORES)], 0)
    zi = np.concatenate([res.results[c]["zi_out"] for c in range(N_CORES)], 0)
    halt = np.concatenate([res.results[c]["halt_out"] for c in range(N_CORES)], 0)
    stack = np.concatenate([res.results[c]["stack_out"] for c in range(N_CORES)], 0)
    return zr, zi, halt, stack
